# revision 3
# baseline (speedup 1.0000x reference)
"""Trainium2 Bass kernel for ConditionalLinearAttention.

Math (per batch element b, shapes hardcoded):
  xf  = x[b].reshape(256, 4096)
  cf  = cond_emb[b].reshape(512, 128)
  kv  = Wcond @ cf                      # (1024, 128)
  k   = softmax(kv[:512], per-row over the 128 cond positions)
  v   = kv[512:]
  ctx[h] = k_h @ v_h.T                  # (64, 64) per head h
  out = Wout @ apply(ctx) @ Wq @ xf + b_out

The whole attention folds into one per-batch matrix
W_comb = Wout @ ctxE @ Wq (256x256); the spatial dimension then sees ONE
(256x256)@(256x4096) GEMM. Sharding: data-parallel over batch, one batch
element per core.

v2 (this file): the kernel is HBM-byte-bound, so everything I/O is bf16
and every DMA row is >=2KB via host-side partition-contiguous packing:
  pk1[p, 1152j+c]   = [cf | WcondT] row (j*128+p), col c   (one 1.2MB DMA)
  pk2[p, 512j+c]    = [Wq | WoutT]  row (j*128+p), col c   (one 0.5MB DMA)
  x_t[p, 4096ck+n]  = xf row (ck*128+p), col n             (8 chunk DMAs)
  out[p, 4096mo+n]  = OUT row (mo*128+p), col n            (4 wave DMAs)
Softmax normalization is folded into the context rows so exp stays
un-normalized and no on-chip transpose is needed. Phase-2 runs 4 waves of
1024 spatial columns, loop-ordered (mo, ck, sub) so the PE stationary
operand is reused across sub-tiles; waves overlap the x chunk stream.
Junk matmuls keep the PE HAM clock gate at 8/8 (2.4 GHz) through the
input-DMA window and the framework teardown.
"""

import os

import numpy as np

B = 8
C = 256
N_SPATIAL = 4096  # 64*64
P = 128
N_CORES = 8

WARM_START = int(os.environ.get("KERNEL_WARM_START", "6"))
WARM_MID = int(os.environ.get("KERNEL_WARM_MID", "2"))
WARM_END = int(os.environ.get("KERNEL_WARM_END", "2"))
SPLIT_PK1 = int(os.environ.get("KERNEL_SPLIT_PK1", "0"))

_CACHE = {}
LAST_RESULTS = None  # BassKernelResults of the most recent run (for test.py)


def _build_nc():
    import concourse.bacc as bacc
    import concourse.mybir as mybir
    import concourse.tile as tile

    fp32 = mybir.dt.float32
    bf16 = mybir.dt.bfloat16
    AF = mybir.ActivationFunctionType

    nc = bacc.Bacc("TRN2", target_bir_lowering=False, debug=False,
                   num_devices=N_CORES)

    # Host-packed, partition-contiguous layouts (see module docstring).
    x_t = nc.dram_tensor("x", [P, 2 * N_SPATIAL], bf16, kind="ExternalInput").ap()
    pk1_t = nc.dram_tensor("pk1", [P, 4 * 1152], bf16, kind="ExternalInput").ap()
    pk2_t = nc.dram_tensor("pk2", [P, 4 * 512], bf16, kind="ExternalInput").ap()
    bias_t = nc.dram_tensor("bias", [256, 1], fp32, kind="ExternalInput").ap()
    out_t = nc.dram_tensor("out", [P, 2 * N_SPATIAL], bf16, kind="ExternalOutput").ap()

    NW = 512          # matmul moving width
    WAVE = 1024       # phase-2 columns per output wave
    NWAVES = N_SPATIAL // WAVE

    with tile.TileContext(nc) as tc:
        with (
            tc.tile_pool(name="main", bufs=1) as mainp,
            tc.tile_pool(name="work", bufs=2) as workp,
            tc.tile_pool(name="outp", bufs=3) as outp,
            tc.tile_pool(name="ps", bufs=3, space="PSUM") as psp,
            tc.tile_pool(name="psO", bufs=5, space="PSUM") as psO,
        ):
            br = bias_t.rearrange("(mo p) one -> p mo one", p=P)   # (128, 2, 1)
            outr = out_t.rearrange("p (mo n) -> p mo n", mo=2)     # (128, 2, 4096)

            # --- input DMAs, critical-path order on the sync HWDGE ring
            if SPLIT_PK1:
                pk1_sb = mainp.tile([P, 4 * 1152], bf16)
                for j in range(4):
                    nc.sync.dma_start(pk1_sb[:, 1152 * j:1152 * (j + 1)],
                                      pk1_t[:, 1152 * j:1152 * (j + 1)])
            else:
                pk1_sb = mainp.tile([P, 4 * 1152], bf16)
                nc.sync.dma_start(pk1_sb, pk1_t)
            pk2_sb = mainp.tile([P, 4 * 512], bf16)
            nc.sync.dma_start(pk2_sb, pk2_t)
            x_sb = []
            for cc in range(2 * NWAVES):   # order: w0ck0, w0ck1, w1ck0, ...
                w, ck = divmod(cc, 2)
                t = mainp.tile([P, WAVE], bf16, tag=f"x{cc}")
                nc.sync.dma_start(
                    t, x_t[:, 4096 * ck + WAVE * w: 4096 * ck + WAVE * (w + 1)])
                x_sb.append(t)
            # bias: 256 tiny strided descriptors -> keep off the sync ring
            bias_sb = mainp.tile([P, 2, 1], fp32)
            nc.gpsimd.dma_start(bias_sb, br)

            # warmup operand tiles + small constants
            wl = mainp.tile([P, P], bf16)
            nc.gpsimd.memset(wl, 0.0)
            ctx_bd = mainp.tile([P, 4, P], bf16)
            nc.gpsimd.memset(ctx_bd, 0.0)
            wz = mainp.tile([P, NW], bf16)
            nc.vector.memset(wz, 0.0)
            ones_sb = mainp.tile([P, 1], fp32)
            nc.vector.memset(ones_sb, 1.0)

            # PE warmup: junk matmuls with no DMA deps fill the otherwise-idle
            # input-DMA window so HAM unthrottles (1.2 -> 2.4 GHz) before the
            # real matmuls start
            def keep_warm(n):
                for _ in range(n):
                    pj = psO.tile([P, NW], fp32, tag="O")
                    nc.tensor.matmul(pj, wl, wz, start=True, stop=True)

            keep_warm(WARM_START)

            # --- phase 1: per-batch W_comb (256x256) ---
            # kvT (cond position m on partitions): k half and v half
            pkv = psp.tile([P, NW], fp32, tag="p1")
            for j in range(4):
                nc.tensor.matmul(pkv, pk1_sb[:, 1152 * j:1152 * j + 128],
                                 pk1_sb[:, 1152 * j + 128:1152 * j + 640],
                                 start=(j == 0), stop=(j == 3))
            pvv = psp.tile([P, NW], fp32, tag="p1")
            for j in range(4):
                nc.tensor.matmul(pvv, pk1_sb[:, 1152 * j:1152 * j + 128],
                                 pk1_sb[:, 1152 * j + 640:1152 * (j + 1)],
                                 start=(j == 0), stop=(j == 3))
            expkT = mainp.tile([P, NW], fp32)
            nc.scalar.activation(out=expkT, in_=pkv, func=AF.Exp)
            vT = mainp.tile([P, NW], fp32)
            nc.vector.tensor_copy(out=vT, in_=pvv)

            # softmax denominators as columns: Z[hd] = sum_m expkT[m, hd]
            pz = psp.tile([P, 4], fp32, tag="p1")
            for i in range(4):
                nc.tensor.matmul(pz[:, i:i + 1], expkT[:, 128 * i:128 * (i + 1)],
                                 ones_sb, start=True, stop=True)
            rc = workp.tile([P, 4], fp32)
            nc.vector.reciprocal(rc, pz)
            keep_warm(WARM_MID)

            # per-head-pair context; scale rows by 1/Z while extracting the
            # diagonal 64x64 blocks into the block-diagonal layout
            pc = psp.tile([P, 4, 128], fp32, tag="p1")
            for i in range(4):
                nc.tensor.matmul(pc[:, i, :], expkT[:, 128 * i:128 * (i + 1)],
                                 vT[:, 128 * i:128 * (i + 1)], start=True, stop=True)
            for i in range(4):
                nc.vector.tensor_scalar_mul(ctx_bd[0:64, i, 0:64],
                                            pc[0:64, i, 0:64], rc[0:64, i:i + 1])
                nc.vector.tensor_scalar_mul(ctx_bd[64:128, i, 64:128],
                                            pc[64:128, i, 64:128], rc[64:128, i:i + 1])

            # A[he, c] = blockdiag(ctx).T @ Wq  (k-tile i = head pair i)
            paA = psp.tile([P, 2, 256], fp32, tag="p1")
            paB = psp.tile([P, 2, 256], fp32, tag="p1")
            for i in range(4):
                pa = paA[:, i, :] if i < 2 else paB[:, i - 2, :]
                nc.tensor.matmul(pa, ctx_bd[:, i, :],
                                 pk2_sb[:, 512 * i:512 * i + 256],
                                 start=True, stop=True)
            A_sb = mainp.tile([P, 4, 256], bf16)
            nc.scalar.activation(out=A_sb[:, 0:2, :], in_=paA, func=AF.Copy)
            nc.scalar.activation(out=A_sb[:, 2:4, :], in_=paB, func=AF.Copy)

            # W_combT[c, o] = sum_he A[he, c] * WoutT[he, o]
            pw = psp.tile([P, 2, 256], fp32, tag="p1")
            for mc in range(2):
                for kk in range(4):
                    nc.tensor.matmul(pw[:, mc, :],
                                     A_sb[:, kk, 128 * mc:128 * (mc + 1)],
                                     pk2_sb[:, 512 * kk + 256:512 * (kk + 1)],
                                     start=(kk == 0), stop=(kk == 3))
            wc_sb = mainp.tile([P, 2, 256], bf16)
            nc.vector.tensor_copy(out=wc_sb, in_=pw)

            # --- phase 2: OUT = W_comb @ xf + bias, 4 waves of 1024 columns
            # loop order (mo, ck, sub) reuses the PE stationary operand
            for w in range(NWAVES):
                xcA, xcB = x_sb[2 * w], x_sb[2 * w + 1]
                ot = outp.tile([P, 2, WAVE], bf16, tag="osb")
                ps = [psO.tile([P, NW], fp32, tag="O", name=f"psO_w{w}_{k}")
                      for k in range(4)]
                for mo in range(2):
                    for ck in range(2):
                        xc = xcA if ck == 0 else xcB
                        for sub in range(2):
                            nc.tensor.matmul(
                                ps[2 * mo + sub],
                                wc_sb[:, ck, 128 * mo:128 * (mo + 1)],
                                xc[:, NW * sub:NW * (sub + 1)],
                                start=(ck == 0), stop=(ck == 1))
                for sub in range(2):
                    nc.scalar.activation(out=ot[:, 0, NW * sub:NW * (sub + 1)],
                                         in_=ps[sub], func=AF.Identity,
                                         bias=bias_sb[:, 0, :], scale=1.0)
                    nc.vector.tensor_scalar_add(out=ot[:, 1, NW * sub:NW * (sub + 1)],
                                                in0=ps[2 + sub],
                                                scalar1=bias_sb[:, 1, :])
                nc.scalar.dma_start(outr[:, :, WAVE * w:WAVE * (w + 1)], ot)

            keep_warm(WARM_END)

    nc.compile()
    return nc


def kernel(x, cond_emb, Wq, Wcond, Wout, b_out):
    from concourse.bass_utils import run_bass_kernel_spmd
    import ml_dtypes

    global LAST_RESULTS

    if "nc" not in _CACHE:
        _CACHE["nc"] = _build_nc()
    nc = _CACHE["nc"]

    bf = ml_dtypes.bfloat16
    # x: (8,256,64,64) -> per-batch [128, 2*4096] partition-contiguous
    xf = np.asarray(x, np.float32).reshape(B, 2, P, N_SPATIAL)
    xp = np.ascontiguousarray(xf.transpose(0, 2, 1, 3)).reshape(B, P, 2 * N_SPATIAL)
    xp = xp.astype(bf)
    # pk1: [cf | WcondT] (512, 1152) -> [128, 4*1152]
    cf = np.asarray(cond_emb, np.float32).reshape(B, 512, 128)
    wcondT = np.ascontiguousarray(np.asarray(Wcond, np.float32).T)  # (512, 1024)
    pk1 = np.empty((B, 512, 1152), np.float32)
    pk1[:, :, 0:128] = cf
    pk1[:, :, 128:1152] = wcondT[None]
    pk1 = np.ascontiguousarray(
        pk1.reshape(B, 4, P, 1152).transpose(0, 2, 1, 3)).reshape(B, P, 4 * 1152)
    pk1 = pk1.astype(bf)
    # pk2: [Wq | WoutT] (512, 512) -> [128, 4*512] (same for all cores)
    pk2 = np.concatenate([np.asarray(Wq, np.float32),
                          np.ascontiguousarray(np.asarray(Wout, np.float32).T)],
                         axis=1)
    pk2 = np.ascontiguousarray(
        pk2.reshape(4, P, 512).transpose(1, 0, 2)).reshape(P, 4 * 512).astype(bf)
    bias = np.ascontiguousarray(np.asarray(b_out, np.float32).reshape(256, 1))

    in_maps = [
        {
            "x": np.ascontiguousarray(xp[b]),
            "pk1": np.ascontiguousarray(pk1[b]),
            "pk2": pk2,
            "bias": bias,
        }
        for b in range(B)
    ]

    trace = bool(int(os.environ.get("KERNEL_TRACE", "0")))
    res = run_bass_kernel_spmd(nc, in_maps, core_ids=list(range(N_CORES)),
                               trace=trace)
    LAST_RESULTS = res
    # out[p, 4096*mo + n] = OUT[mo*128+p, n]
    out = np.stack([np.asarray(res.results[b]["out"]) for b in range(B)])
    out = out.reshape(B, P, 2, N_SPATIAL).transpose(0, 2, 1, 3)
    return np.ascontiguousarray(out).reshape(B, C, 64, 64).astype(np.float32)


if __name__ == "__main__":
    xs = np.random.RandomState(0)
    ins = {
        "x": xs.randn(8, 256, 64, 64).astype(np.float32),
        "cond_emb": xs.randn(8, 512, 1, 128).astype(np.float32),
        "Wq": (xs.randn(512, 256) * 0.05).astype(np.float32),
        "Wcond": (xs.randn(1024, 512) * 0.05).astype(np.float32),
        "Wout": (xs.randn(256, 512) * 0.05).astype(np.float32),
        "b_out": np.zeros(256, np.float32),
    }
    o = kernel(**ins)
    print("ran, shape", o.shape)
